# revision 1
# baseline (speedup 1.0000x reference)
"""Bass/Trainium2 kernel for nn_CrossAttentionLayer.

out = softmax((x_q Wq^T + bq)(x_k Wk^T + bk)^T) (x_v Wv^T + bv)

Sharding: data-parallel over batch B=8 across the 8 NeuronCores.

Exact-math refactorings (all associativity / softmax invariances):
  - bk drops out of softmax (per-row constant in the logits).
  - logits = x_q (Wq^T Wk) x_k^T + (x_k @ (bq Wk))^T : the two weight
    matrices fold into one host-precomputed M = Wq^T Wk, and the bq
    cross term becomes a per-key scalar ku fused into the exp bias.
    This removes the K projection from the device entirely.
  - attn @ (x_v Wv^T) = (attn @ x_v) Wv^T : the V projection moves
    AFTER the attention-weighted sum, so the device consumes raw x_v
    (no V projection pass, no on-chip transposes at all).
  - bv is added on the host (softmax rows sum to 1).
  - softmax normalization commutes with the PV matmul: device returns
    unnormalized (attn@xv)Wv^T transposed, plus row-sums; host divides.

Precision split: the score path (t = x_q M, x_k) stays fp32r — exp
amplifies logit rounding (logit std ~11 here), bf16 there costs ~1e-2
rel err. The post-softmax path (exp weights, raw x_v, Wv) is bf16:
those errors stay at the ~0.4% level and partially cancel through the
softmax normalization. Accumulation is always fp32 in PSUM.

Device layout: everything is produced directly in its consumer's
layout ([e, n] / [k, n] tiles), so there are no on-chip transposes and
every DMA moves >=2KB contiguous lines. t takes a DRAM round-trip
(SBUF cannot hold both fp32r score operands plus residents).
"""

import sys

if "/opt/trn_rl_repo" not in sys.path:
    sys.path.insert(0, "/opt/trn_rl_repo")

import numpy as np

B = 8          # batch == number of cores
D = 1024       # model/latent dim
N = 2048       # tokens (queries == keys)
P = 128        # partitions
DC = D // P    # 8 chunks of the d/e axis
JT = N // P    # 16 key tiles
F = 512        # matmul moving free dim (fp32 PSUM max)
NB = N // F    # 4 query blocks

_CACHE = {}


def _build_nc():
    import concourse.bass as bass
    import concourse.mybir as mybir
    import concourse.tile as tile
    from concourse import bacc
    from contextlib import ExitStack

    f32 = mybir.dt.float32
    f32r = mybir.dt.float32r
    bf16 = mybir.dt.bfloat16
    EXP = mybir.ActivationFunctionType.Exp

    nc = bacc.Bacc("TRN2", target_bir_lowering=False, debug=False, num_devices=B)

    # DRAM inputs
    mqk = nc.dram_tensor("mqk", [D, D], f32r, kind="ExternalInput").ap()
    xqt = nc.dram_tensor("xqt", [D, N], f32r, kind="ExternalInput").ap()
    xkt = nc.dram_tensor("xkt", [D, N], f32r, kind="ExternalInput").ap()
    xv = nc.dram_tensor("xv", [N, D], bf16, kind="ExternalInput").ap()
    wvt = nc.dram_tensor("wvt", [D, D], bf16, kind="ExternalInput").ap()
    kub = nc.dram_tensor("kub", [P, JT], f32, kind="ExternalInput").ap()

    t_int = nc.dram_tensor("t_int", [D, N], f32r).ap()

    acct = nc.dram_tensor("acct", [D, N], f32, kind="ExternalOutput").ap()
    rowsum = nc.dram_tensor("rowsum", [NB, F], f32, kind="ExternalOutput").ap()

    with ExitStack() as ctx:
        tc = ctx.enter_context(tile.TileContext(nc))
        # Persistent pools (live for the whole kernel)
        ktp = ctx.enter_context(tc.tile_pool(name="ktp", bufs=1))
        t0p = ctx.enter_context(tc.tile_pool(name="t0p", bufs=1))
        vp = ctx.enter_context(tc.tile_pool(name="vp", bufs=1))
        wvp = ctx.enter_context(tc.tile_pool(name="wvp", bufs=1))
        cst = ctx.enter_context(tc.tile_pool(name="cst", bufs=1))

        ones = cst.tile([P, 1], bf16, name="ones", tag="c_ones")
        nc.vector.memset(ones, 1.0)
        ku_sb = cst.tile([P, JT], f32, name="ku_sb", tag="c_ku")
        nc.sync.dma_start(out=ku_sb, in_=kub)

        # Persistent SBUF residents
        kt_sb = ktp.tile([P, DC, N], f32r, name="kt_sb", tag="kt")  # 64KB/part
        v_sb = vp.tile([P, JT, D], bf16, name="v_sb", tag="v")      # 32KB/part
        wv_sb = wvp.tile([P, DC, D], bf16, name="wv_sb", tag="wv")  # 16KB/part

        xktr = xkt.rearrange("(c p) n -> p c n", p=P)
        xvr = xv.rearrange("(t p) d -> p t d", p=P)
        wvtr = wvt.rearrange("(c p) e -> p c e", p=P)
        t_r = t_int.rearrange("(c p) n -> p c n", p=P)

        with ExitStack() as bctx:
            # Phase-B-only pools (freed before phase C pools allocate)
            mp = bctx.enter_context(tc.tile_pool(name="mp", bufs=1))
            xqp = bctx.enter_context(tc.tile_pool(name="xqp", bufs=2))
            top = bctx.enter_context(tc.tile_pool(name="top", bufs=4))
            psb = bctx.enter_context(tc.tile_pool(name="psb", bufs=4, space="PSUM"))

            m_sb = mp.tile([P, DC, D], f32r, name="m_sb", tag="m")      # 32KB

            mr = mqk.rearrange("(c p) e -> p c e", p=P)
            xqtr = xqt.rearrange("(c p) n -> p c n", p=P)

            # Load order: phase-B inputs (m, xq0) in fine-grained chunks so
            # the first matmul group starts after ~1MB, then the phase-C
            # residents interleaved in ~1-2MB pieces across DMA queues so
            # later xq stream loads are not stuck behind them.
            xq0 = xqp.tile([P, DC, F], f32r, name="xq_sb", tag="xq")
            for dp in range(DC):
                nc.sync.dma_start(out=m_sb[:, dp, :], in_=mr[:, dp, :])
                nc.sync.dma_start(
                    out=xq0[:, dp, :], in_=xqtr[:, dp, 0:F]
                )
            xq1 = xqp.tile([P, DC, F], f32r, name="xq_sb", tag="xq")
            nc.sync.dma_start(out=xq1, in_=xqtr[:, :, F:2 * F])
            xq2 = xqp.tile([P, DC, F], f32r, name="xq_sb", tag="xq")
            nc.sync.dma_start(out=xq2, in_=xqtr[:, :, 2 * F:3 * F])
            xq3 = xqp.tile([P, DC, F], f32r, name="xq_sb", tag="xq")
            nc.sync.dma_start(out=xq3, in_=xqtr[:, :, 3 * F:4 * F])
            # Residents in ~512KB pieces: phase B's rolling t_r
            # write-backs interleave fairly into the DMA FIFO instead of
            # queueing behind three multi-MB transfers (which starves the
            # 3-deep t staging pool and stalls the B pipeline).
            for c in range(DC):
                for h in range(2):
                    nc.sync.dma_start(
                        out=kt_sb[:, c, h * 1024:(h + 1) * 1024],
                        in_=xktr[:, c, h * 1024:(h + 1) * 1024],
                    )
            for j in range(0, JT, 2):
                nc.sync.dma_start(out=v_sb[:, j:j + 2, :], in_=xvr[:, j:j + 2, :])
            for c in range(0, DC, 2):
                nc.sync.dma_start(
                    out=wv_sb[:, c:c + 2, :], in_=wvtr[:, c:c + 2, :]
                )

            # Block 0's t tiles stay SBUF-resident (16KB/part): phase C
            # block 0 then needs no DRAM round-trip, so scores start the
            # moment phase B drains instead of waiting on a tq readback
            # queued behind the resident loads.
            t0stage = [
                t0p.tile([P, F], f32r, name=f"t0_{ec}", tag=f"t0_{ec}")
                for ec in range(DC)
            ]

            # ---- Phase B: tT[e, n] = (x_q M)^T -> DRAM (fp32r) ----
            for t in range(NB):
                if t == 0:
                    xq_sb = xq0
                elif t == 1:
                    xq_sb = xq1
                elif t == 2:
                    xq_sb = xq2
                else:
                    xq_sb = xq3
                for ec in range(DC):
                    ps = psb.tile([P, F], f32, name="ps_t", tag="psb")
                    for dp in range(DC):
                        nc.tensor.matmul(
                            ps,
                            lhsT=m_sb[:, dp, ec * P:(ec + 1) * P],
                            rhs=xq_sb[:, dp, :],
                            start=(dp == 0),
                            stop=(dp == DC - 1),
                        )
                    if t == 0:
                        nc.vector.tensor_copy(t0stage[ec], ps)
                    else:
                        to = top.tile([P, F], f32r, name="to_t", tag="to")
                        nc.vector.tensor_copy(to, ps)
                        nc.scalar.dma_start(
                            out=t_r[:, ec, t * F:(t + 1) * F], in_=to
                        )

        # ---- Phase C: per 512-query block: scores^T, exp, rowsum, PV, Wv ----
        tqp = ctx.enter_context(tc.tile_pool(name="tqp", bufs=2))
        exp_ = ctx.enter_context(tc.tile_pool(name="exp", bufs=1))
        rsp = ctx.enter_context(tc.tile_pool(name="rsp", bufs=2))
        otp = ctx.enter_context(tc.tile_pool(name="otp", bufs=3))
        psm = ctx.enter_context(tc.tile_pool(name="psm", bufs=2, space="PSUM"))
        psv = ctx.enter_context(tc.tile_pool(name="psv", bufs=2, space="PSUM"))
        pso = ctx.enter_context(tc.tile_pool(name="pso", bufs=2, space="PSUM"))
        psr = ctx.enter_context(tc.tile_pool(name="psr", bufs=1, space="PSUM"))

        for t in range(NB):
            if t == 0:
                tq = None
            else:
                tq = tqp.tile([P, DC, F], f32r, name="tq", tag="tq")
                nc.sync.dma_start(out=tq, in_=t_r[:, :, t * F:(t + 1) * F])
            ex = exp_.tile([P, JT, F], bf16, name="ex", tag="ex")
            rp = psr.tile([1, F], f32, name="rp", tag="psr")
            for jt in range(JT):
                ps = psm.tile([P, F], f32, name="ps_s", tag="psm")
                for ec in range(DC):
                    nc.tensor.matmul(
                        ps,
                        lhsT=kt_sb[:, ec, jt * P:(jt + 1) * P],
                        rhs=t0stage[ec] if t == 0 else tq[:, ec, :],
                        start=(ec == 0),
                        stop=(ec == DC - 1),
                    )
                # ex = exp(scoresT + ku[key]) — per-partition bias fused
                nc.scalar.activation(ex[:, jt, :], ps, EXP,
                                     bias=ku_sb[:, jt:jt + 1])
            # Rowsums batched after the score loop: the ones lhsT loads
            # once for 16 consecutive matmuls instead of 16 times.
            for jt in range(JT):
                nc.tensor.matmul(
                    rp,
                    lhsT=ones,
                    rhs=ex[:, jt, :],
                    start=(jt == 0),
                    stop=(jt == JT - 1),
                    skip_group_check=True,
                )
            rs = otp.tile([1, F], f32, name="rs", tag="otrs")
            nc.vector.tensor_copy(rs, rp)
            nc.scalar.dma_start(out=rowsum[t:t + 1, :], in_=rs)

            # rT[d, q] = sum_k xv[k, d] * ex[k, q]
            rt = rsp.tile([P, DC, F], bf16, name="rt", tag="rs")
            for dc in range(DC):
                pv = psv.tile([P, F], f32, name="pv", tag="psv")
                for jt in range(JT):
                    nc.tensor.matmul(
                        pv,
                        lhsT=v_sb[:, jt, dc * P:(dc + 1) * P],
                        rhs=ex[:, jt, :],
                        start=(jt == 0),
                        stop=(jt == JT - 1),
                    )
                nc.vector.tensor_copy(rt[:, dc, :], pv)

            # outT[e, q] = sum_d Wv[e, d] * rT[d, q]
            for ec in range(DC):
                po = pso.tile([P, F], f32, name="po", tag="pso")
                for dp in range(DC):
                    nc.tensor.matmul(
                        po,
                        lhsT=wv_sb[:, dp, ec * P:(ec + 1) * P],
                        rhs=rt[:, dp, :],
                        start=(dp == 0),
                        stop=(dp == DC - 1),
                    )
                ot = otp.tile([P, F], f32, name="ot", tag="ot")
                nc.vector.tensor_copy(ot, po)
                nc.scalar.dma_start(
                    out=acct[ec * P:(ec + 1) * P, t * F:(t + 1) * F], in_=ot
                )

    nc.compile()
    return nc


def get_nc():
    if "nc" not in _CACHE:
        _CACHE["nc"] = _build_nc()
    return _CACHE["nc"]


def _bf16():
    import concourse.mybir as mybir
    return mybir.dt.np(mybir.dt.bfloat16)


def make_in_maps(query, key, value, Wq, bq, Wk, bk, Wv, bv):
    bf16 = _bf16()
    query = np.asarray(query, dtype=np.float32)
    key = np.asarray(key, dtype=np.float32)
    value = np.asarray(value, dtype=np.float32)
    Wq64 = np.asarray(Wq, dtype=np.float64)
    Wk64 = np.asarray(Wk, dtype=np.float64)
    # M = Wq^T Wk  (exact on host);  u = bq @ Wk;  ku[n] = x_k[n] . u
    M = (Wq64.T @ Wk64).astype(np.float32)
    u = np.asarray(bq, dtype=np.float64) @ Wk64
    mqk = np.ascontiguousarray(M)
    wvt = np.ascontiguousarray(np.asarray(Wv, dtype=np.float32).T.astype(bf16))
    in_maps = []
    for b in range(B):
        ku = (key[b].astype(np.float64) @ u).astype(np.float32)  # [N]
        in_maps.append(
            {
                "mqk": mqk,
                "xqt": np.ascontiguousarray(query[b].T),
                "xkt": np.ascontiguousarray(key[b].T),
                "xv": np.ascontiguousarray(value[b].astype(bf16)),
                "wvt": wvt,
                "kub": np.ascontiguousarray(ku.reshape(JT, P).T),
            }
        )
    return in_maps


def postprocess(results, bv):
    bv = np.asarray(bv, dtype=np.float32)
    outs = []
    for b in range(B):
        acct = results[b]["acct"]               # [D, N] unnormalized out^T
        rsum = results[b]["rowsum"].reshape(N)  # [N] softmax denominators
        outs.append(acct.T / rsum[:, None] + bv[None, :])
    return np.stack(outs).astype(np.float32)


def kernel(query, key, value, Wq, bq, Wk, bk, Wv, bv):
    from concourse.bass_utils import run_bass_kernel_spmd

    nc = get_nc()
    in_maps = make_in_maps(query, key, value, Wq, bq, Wk, bk, Wv, bv)
    res = run_bass_kernel_spmd(nc, in_maps, list(range(B)))
    return postprocess(res.results, bv)



# revision 2
# speedup vs baseline: 13.0224x; 13.0224x over previous
"""Bass/Trainium2 kernel for nn_CrossAttentionLayer.

out = softmax((x_q Wq^T + bq)(x_k Wk^T + bk)^T) (x_v Wv^T + bv)

Sharding: data-parallel over batch B=8 across the 8 NeuronCores.

Exact-math refactorings (all associativity / softmax invariances):
  - bk drops out of softmax (per-row constant in the logits).
  - logits = x_q (Wq^T Wk) x_k^T + (x_k @ (bq Wk))^T : the two weight
    matrices fold into one host-precomputed M = Wq^T Wk, and the bq
    cross term becomes a per-key scalar ku fused into the exp bias.
    This removes the K projection from the device entirely.
  - attn @ (x_v Wv^T) = (attn @ x_v) Wv^T : the V projection moves
    AFTER the attention-weighted sum, so the device consumes raw x_v
    (no V projection pass, no on-chip transposes at all).
  - bv is added on the host (softmax rows sum to 1).
  - softmax normalization commutes with the PV matmul: device returns
    unnormalized (attn@xv)Wv^T transposed, plus row-sums; host divides.

Precision split: the score path (t = x_q M, x_k) stays fp32r — exp
amplifies logit rounding (logit std ~11 here), bf16 there costs ~1e-2
rel err. The post-softmax path (exp weights, raw x_v, Wv) is bf16:
those errors stay at the ~0.4% level and partially cancel through the
softmax normalization. Accumulation is always fp32 in PSUM.

Device layout: everything is produced directly in its consumer's
layout ([e, n] / [k, n] tiles), so there are no on-chip transposes and
every DMA moves >=2KB contiguous lines. t takes a DRAM round-trip
(SBUF cannot hold both fp32r score operands plus residents).

build_nc(repeats=R) additionally emits the SAME kernel body R times
back-to-back in one program (each pass is the complete kernel: all
input DMAs from DRAM, all compute, all output DMAs; pass r uses its
own internal t scratch). kernel() and grading always use repeats=1;
the R-pass program exists so test.py can measure per-execution device
time differentially ((T_R - T_1)/(R-1)), cancelling the multi-ms
per-dispatch tunnel overhead that a single-call wall-clock includes.
"""

import sys

if "/opt/trn_rl_repo" not in sys.path:
    sys.path.insert(0, "/opt/trn_rl_repo")

import numpy as np

B = 8          # batch == number of cores
D = 1024       # model/latent dim
N = 2048       # tokens (queries == keys)
P = 128        # partitions
DC = D // P    # 8 chunks of the d/e axis
JT = N // P    # 16 key tiles
F = 512        # matmul moving free dim (fp32 PSUM max)
NB = N // F    # 4 query blocks

_CACHE = {}


def build_nc(repeats=1):
    import concourse.bass as bass
    import concourse.mybir as mybir
    import concourse.tile as tile
    from concourse import bacc
    from contextlib import ExitStack

    f32 = mybir.dt.float32
    f32r = mybir.dt.float32r
    bf16 = mybir.dt.bfloat16
    EXP = mybir.ActivationFunctionType.Exp

    nc = bacc.Bacc("TRN2", target_bir_lowering=False, debug=False, num_devices=B)

    # DRAM inputs
    mqk = nc.dram_tensor("mqk", [D, D], f32r, kind="ExternalInput").ap()
    xqt = nc.dram_tensor("xqt", [D, N], f32r, kind="ExternalInput").ap()
    xkt = nc.dram_tensor("xkt", [D, N], f32r, kind="ExternalInput").ap()
    xv = nc.dram_tensor("xv", [N, D], bf16, kind="ExternalInput").ap()
    wvt = nc.dram_tensor("wvt", [D, D], bf16, kind="ExternalInput").ap()
    kub = nc.dram_tensor("kub", [P, JT], f32, kind="ExternalInput").ap()

    t_ints = [nc.dram_tensor(f"t_int{r}", [D, N], f32r).ap() for r in range(repeats)]

    acct = nc.dram_tensor("acct", [D, N], f32, kind="ExternalOutput").ap()
    rowsum = nc.dram_tensor("rowsum", [NB, F], f32, kind="ExternalOutput").ap()

    with ExitStack() as octx:
        tc = octx.enter_context(tile.TileContext(nc))
        for r in range(repeats):
            _emit_pass(nc, tc, mybir, r, mqk, xqt, xkt, xv, wvt, kub,
                       t_ints[r], acct, rowsum)

    nc.compile()
    return nc


def _emit_pass(nc, tc, mybir, r, mqk, xqt, xkt, xv, wvt, kub, t_int, acct, rowsum):
    import concourse.mybir as _mybir
    from contextlib import ExitStack

    f32 = mybir.dt.float32
    f32r = mybir.dt.float32r
    bf16 = mybir.dt.bfloat16
    EXP = mybir.ActivationFunctionType.Exp
    sfx = f"_r{r}"

    with ExitStack() as ctx:
        # Pools living for the whole pass
        ktp = ctx.enter_context(tc.tile_pool(name=f"ktp{sfx}", bufs=1))
        t0p = ctx.enter_context(tc.tile_pool(name=f"t0p{sfx}", bufs=1))
        vp = ctx.enter_context(tc.tile_pool(name=f"vp{sfx}", bufs=1))
        wvp = ctx.enter_context(tc.tile_pool(name=f"wvp{sfx}", bufs=1))
        cst = ctx.enter_context(tc.tile_pool(name=f"cst{sfx}", bufs=1))

        ones = cst.tile([P, 1], bf16, name="ones", tag=f"c_ones{sfx}")
        nc.vector.memset(ones, 1.0)
        ku_sb = cst.tile([P, JT], f32, name="ku_sb", tag=f"c_ku{sfx}")
        nc.sync.dma_start(out=ku_sb, in_=kub)

        # Persistent SBUF residents
        kt_sb = ktp.tile([P, DC, N], f32r, name="kt_sb", tag=f"kt{sfx}")  # 64KB/part
        v_sb = vp.tile([P, JT, D], bf16, name="v_sb", tag=f"v{sfx}")      # 32KB/part
        wv_sb = wvp.tile([P, DC, D], bf16, name="wv_sb", tag=f"wv{sfx}")  # 16KB/part

        xktr = xkt.rearrange("(c p) n -> p c n", p=P)
        xvr = xv.rearrange("(t p) d -> p t d", p=P)
        wvtr = wvt.rearrange("(c p) e -> p c e", p=P)
        t_r = t_int.rearrange("(c p) n -> p c n", p=P)

        with ExitStack() as bctx:
            # Phase-B-only pools (freed before phase C pools allocate)
            mp = bctx.enter_context(tc.tile_pool(name=f"mp{sfx}", bufs=1))
            xqp = bctx.enter_context(tc.tile_pool(name=f"xqp{sfx}", bufs=2))
            top = bctx.enter_context(tc.tile_pool(name=f"top{sfx}", bufs=4))
            psb = bctx.enter_context(tc.tile_pool(name=f"psb{sfx}", bufs=4, space="PSUM"))

            m_sb = mp.tile([P, DC, D], f32r, name="m_sb", tag=f"m{sfx}")  # 32KB

            mr = mqk.rearrange("(c p) e -> p c e", p=P)
            xqtr = xqt.rearrange("(c p) n -> p c n", p=P)

            # Load order: phase-B inputs (m, xq0) in fine-grained chunks so
            # the first matmul group starts after ~1MB, then the phase-C
            # residents interleaved in ~1-2MB pieces across DMA queues so
            # later xq stream loads are not stuck behind them.
            xq0 = xqp.tile([P, DC, F], f32r, name="xq_sb", tag=f"xq{sfx}")
            for dp in range(DC):
                nc.sync.dma_start(out=m_sb[:, dp, :], in_=mr[:, dp, :])
                nc.sync.dma_start(
                    out=xq0[:, dp, :], in_=xqtr[:, dp, 0:F]
                )
            xq1 = xqp.tile([P, DC, F], f32r, name="xq_sb", tag=f"xq{sfx}")
            nc.sync.dma_start(out=xq1, in_=xqtr[:, :, F:2 * F])
            xq2 = xqp.tile([P, DC, F], f32r, name="xq_sb", tag=f"xq{sfx}")
            nc.sync.dma_start(out=xq2, in_=xqtr[:, :, 2 * F:3 * F])
            xq3 = xqp.tile([P, DC, F], f32r, name="xq_sb", tag=f"xq{sfx}")
            nc.sync.dma_start(out=xq3, in_=xqtr[:, :, 3 * F:4 * F])
            # Residents in ~512KB pieces: phase B's rolling t_r
            # write-backs interleave fairly into the DMA FIFO instead of
            # queueing behind three multi-MB transfers (which starves the
            # 3-deep t staging pool and stalls the B pipeline).
            for c in range(DC):
                for h in range(2):
                    nc.sync.dma_start(
                        out=kt_sb[:, c, h * 1024:(h + 1) * 1024],
                        in_=xktr[:, c, h * 1024:(h + 1) * 1024],
                    )
            for j in range(0, JT, 2):
                nc.sync.dma_start(out=v_sb[:, j:j + 2, :], in_=xvr[:, j:j + 2, :])
            for c in range(0, DC, 2):
                nc.sync.dma_start(
                    out=wv_sb[:, c:c + 2, :], in_=wvtr[:, c:c + 2, :]
                )

            # Block 0's t tiles stay SBUF-resident (16KB/part): phase C
            # block 0 then needs no DRAM round-trip, so scores start the
            # moment phase B drains instead of waiting on a tq readback
            # queued behind the resident loads.
            t0stage = [
                t0p.tile([P, F], f32r, name=f"t0_{ec}", tag=f"t0_{ec}{sfx}")
                for ec in range(DC)
            ]

            # ---- Phase B: tT[e, n] = (x_q M)^T -> DRAM (fp32r) ----
            for t in range(NB):
                if t == 0:
                    xq_sb = xq0
                elif t == 1:
                    xq_sb = xq1
                elif t == 2:
                    xq_sb = xq2
                else:
                    xq_sb = xq3
                for ec in range(DC):
                    ps = psb.tile([P, F], f32, name="ps_t", tag=f"psb{sfx}")
                    for dp in range(DC):
                        nc.tensor.matmul(
                            ps,
                            lhsT=m_sb[:, dp, ec * P:(ec + 1) * P],
                            rhs=xq_sb[:, dp, :],
                            start=(dp == 0),
                            stop=(dp == DC - 1),
                        )
                    if t == 0:
                        nc.vector.tensor_copy(t0stage[ec], ps)
                    else:
                        to = top.tile([P, F], f32r, name="to_t", tag=f"to{sfx}")
                        nc.vector.tensor_copy(to, ps)
                        nc.scalar.dma_start(
                            out=t_r[:, ec, t * F:(t + 1) * F], in_=to
                        )

        # ---- Phase C: per 512-query block: scores^T, exp, rowsum, PV, Wv ----
        tqp = ctx.enter_context(tc.tile_pool(name=f"tqp{sfx}", bufs=2))
        exp_ = ctx.enter_context(tc.tile_pool(name=f"exp{sfx}", bufs=1))
        rsp = ctx.enter_context(tc.tile_pool(name=f"rsp{sfx}", bufs=2))
        otp = ctx.enter_context(tc.tile_pool(name=f"otp{sfx}", bufs=3))
        psm = ctx.enter_context(tc.tile_pool(name=f"psm{sfx}", bufs=2, space="PSUM"))
        psv = ctx.enter_context(tc.tile_pool(name=f"psv{sfx}", bufs=2, space="PSUM"))
        pso = ctx.enter_context(tc.tile_pool(name=f"pso{sfx}", bufs=2, space="PSUM"))
        psr = ctx.enter_context(tc.tile_pool(name=f"psr{sfx}", bufs=1, space="PSUM"))

        for t in range(NB):
            if t == 0:
                tq = None
            else:
                tq = tqp.tile([P, DC, F], f32r, name="tq", tag=f"tq{sfx}")
                nc.sync.dma_start(out=tq, in_=t_r[:, :, t * F:(t + 1) * F])
            ex = exp_.tile([P, JT, F], bf16, name="ex", tag=f"ex{sfx}")
            rp = psr.tile([1, F], f32, name="rp", tag=f"psr{sfx}")
            for jt in range(JT):
                ps = psm.tile([P, F], f32, name="ps_s", tag=f"psm{sfx}")
                for ec in range(DC):
                    nc.tensor.matmul(
                        ps,
                        lhsT=kt_sb[:, ec, jt * P:(jt + 1) * P],
                        rhs=t0stage[ec] if t == 0 else tq[:, ec, :],
                        start=(ec == 0),
                        stop=(ec == DC - 1),
                    )
                # ex = exp(scoresT + ku[key]) — per-partition bias fused
                nc.scalar.activation(ex[:, jt, :], ps, EXP,
                                     bias=ku_sb[:, jt:jt + 1])
            # Rowsums batched after the score loop: the ones lhsT loads
            # once for 16 consecutive matmuls instead of 16 times.
            for jt in range(JT):
                nc.tensor.matmul(
                    rp,
                    lhsT=ones,
                    rhs=ex[:, jt, :],
                    start=(jt == 0),
                    stop=(jt == JT - 1),
                    skip_group_check=True,
                )
            rs = otp.tile([1, F], f32, name="rs", tag=f"otrs{sfx}")
            nc.vector.tensor_copy(rs, rp)
            nc.scalar.dma_start(out=rowsum[t:t + 1, :], in_=rs)

            # rT[d, q] = sum_k xv[k, d] * ex[k, q]
            rt = rsp.tile([P, DC, F], bf16, name="rt", tag=f"rs{sfx}")
            for dc in range(DC):
                pv = psv.tile([P, F], f32, name="pv", tag=f"psv{sfx}")
                for jt in range(JT):
                    nc.tensor.matmul(
                        pv,
                        lhsT=v_sb[:, jt, dc * P:(dc + 1) * P],
                        rhs=ex[:, jt, :],
                        start=(jt == 0),
                        stop=(jt == JT - 1),
                    )
                nc.vector.tensor_copy(rt[:, dc, :], pv)

            # outT[e, q] = sum_d Wv[e, d] * rT[d, q]
            for ec in range(DC):
                po = pso.tile([P, F], f32, name="po", tag=f"pso{sfx}")
                for dp in range(DC):
                    nc.tensor.matmul(
                        po,
                        lhsT=wv_sb[:, dp, ec * P:(ec + 1) * P],
                        rhs=rt[:, dp, :],
                        start=(dp == 0),
                        stop=(dp == DC - 1),
                    )
                ot = otp.tile([P, F], f32, name="ot", tag=f"ot{sfx}")
                nc.vector.tensor_copy(ot, po)
                nc.scalar.dma_start(
                    out=acct[ec * P:(ec + 1) * P, t * F:(t + 1) * F], in_=ot
                )


def get_nc():
    if "nc" not in _CACHE:
        _CACHE["nc"] = build_nc(1)
    return _CACHE["nc"]


def _bf16():
    import concourse.mybir as mybir
    return mybir.dt.np(mybir.dt.bfloat16)


def make_in_maps(query, key, value, Wq, bq, Wk, bk, Wv, bv):
    bf16 = _bf16()
    query = np.asarray(query, dtype=np.float32)
    key = np.asarray(key, dtype=np.float32)
    value = np.asarray(value, dtype=np.float32)
    Wq64 = np.asarray(Wq, dtype=np.float64)
    Wk64 = np.asarray(Wk, dtype=np.float64)
    # M = Wq^T Wk  (exact on host);  u = bq @ Wk;  ku[n] = x_k[n] . u
    M = (Wq64.T @ Wk64).astype(np.float32)
    u = np.asarray(bq, dtype=np.float64) @ Wk64
    mqk = np.ascontiguousarray(M)
    wvt = np.ascontiguousarray(np.asarray(Wv, dtype=np.float32).T.astype(bf16))
    in_maps = []
    for b in range(B):
        ku = (key[b].astype(np.float64) @ u).astype(np.float32)  # [N]
        in_maps.append(
            {
                "mqk": mqk,
                "xqt": np.ascontiguousarray(query[b].T),
                "xkt": np.ascontiguousarray(key[b].T),
                "xv": np.ascontiguousarray(value[b].astype(bf16)),
                "wvt": wvt,
                "kub": np.ascontiguousarray(ku.reshape(JT, P).T),
            }
        )
    return in_maps


def postprocess(results, bv):
    bv = np.asarray(bv, dtype=np.float32)
    outs = []
    for b in range(B):
        acct = results[b]["acct"]               # [D, N] unnormalized out^T
        rsum = results[b]["rowsum"].reshape(N)  # [N] softmax denominators
        outs.append(acct.T / rsum[:, None] + bv[None, :])
    return np.stack(outs).astype(np.float32)


def kernel(query, key, value, Wq, bq, Wk, bk, Wv, bv):
    from concourse.bass_utils import run_bass_kernel_spmd

    nc = get_nc()
    in_maps = make_in_maps(query, key, value, Wq, bq, Wk, bk, Wv, bv)
    res = run_bass_kernel_spmd(nc, in_maps, list(range(B)))
    return postprocess(res.results, bv)
